# revision 1
# baseline (speedup 1.0000x reference)
"""Dense Synthesizer Attention — Trainium2 Bass kernel (v2).

Sharding: data-parallel over batch. B=8 batch elements, 8 NeuronCores,
one batch element per core, zero collectives.

Per-core computation (S=1024, F=512, H=8 heads, dk=64), bf16 matmuls with
fp32 PSUM accumulation; q/value transposed and weights cast to bf16 on host:
    hT  = relu(w1^T @ qT + b1)          [1024, 1024]
    awT = w2^T @ hT + b2                [512, 1024]
    head pair hp: aw_A = awT[0:64, hp, :], aw_B = awT[64:128, hp, :]
      scores: PE ROW-TILED pair — A's K=64 matmuls on array rows 0-63,
      B's on rows 64-127, running concurrently (2x throughput for the
      K=64-starved score matmuls).
      E = exp(scores/8) bf16 (ScalarE, no accum drains)
      rowsums r on DVE (tensor_reduce over each head's E tile)
      rbc[dk, q] = 1/r per column, built by PE transposes of 1/r with a
      stride-0 broadcast AP (one [128,128] block per token tile)
      yT = v^T @ E: PE COL-TILED pair — A's M=64 into PSUM partitions
      0-63, B's into 64-127, concurrent; PSUM->SBUF copy multiplies by
      rbc, so yT lands normalized.
    out = yT^T @ wo + bo2 as one K=512 projection (8 m-tiles x 4 K-chunks);
    bo2 = bo + bv @ wo folded on host (softmax rows sum to 1, so the value
    bias commutes past attention).

One software-pipelined loop over 4 head pairs; scores-pair(q) | exp(q) on
ScalarE | attn_v(q-1)/mlp2(q+1)/rbc(q-1) as PE/DVE filler between score
tiles. ScalarE (the exp stream, ~1 elem/cycle/lane) is the critical
resource; everything else hides under it.
"""

import math

import numpy as np

B, S, F = 8, 1024, 512
H, DK = 8, 64
HID = 2 * F
P = 128

N_CORES = 8

_CACHED_NC = None


def _build_nc():
    from contextlib import ExitStack

    import concourse.mybir as mybir
    import concourse.tile as tile
    from concourse import bacc

    dt = mybir.dt
    f32 = dt.float32
    bf16 = dt.bfloat16
    AF = mybir.ActivationFunctionType
    ALU = mybir.AluOpType
    AX = mybir.AxisListType

    SC = S // P      # 8 token chunks
    FC = F // P      # 4 feature chunks
    KC = HID // P    # 8 hidden chunks
    NS = S // 512    # 2 moving-dim chunks
    NH = H // 2      # 4 head pairs

    nc = bacc.Bacc(
        "TRN2",
        target_bir_lowering=False,
        debug=False,
        num_devices=N_CORES,
    )

    q_d = nc.declare_dram_parameter("qT", [F, S], bf16, isOutput=False)
    v_d = nc.declare_dram_parameter("vT", [F, S], bf16, isOutput=False)
    w1_d = nc.declare_dram_parameter("w1", [F, HID], bf16, isOutput=False)
    w2_d = nc.declare_dram_parameter("w2", [HID, F], bf16, isOutput=False)
    wv_d = nc.declare_dram_parameter("wv", [F, F], bf16, isOutput=False)
    wo_d = nc.declare_dram_parameter("wo", [F, F], bf16, isOutput=False)
    b1_d = nc.declare_dram_parameter("b1r", [P, KC], f32, isOutput=False)
    b2_d = nc.declare_dram_parameter("b2r", [P, FC], f32, isOutput=False)
    bo_d = nc.declare_dram_parameter("bo2b", [P, F], f32, isOutput=False)
    id_d = nc.declare_dram_parameter("ident", [P, P], f32, isOutput=False)
    out_d = nc.declare_dram_parameter("out", [S, F], f32, isOutput=True)

    scale = 1.0 / math.sqrt(DK)

    with ExitStack() as ctx:
        tc = ctx.enter_context(tile.TileContext(nc))

        const = ctx.enter_context(tc.tile_pool(name="const", bufs=1))
        big = ctx.enter_context(tc.tile_pool(name="big", bufs=1))
        epool = ctx.enter_context(tc.tile_pool(name="ep", bufs=4))
        rpool = ctx.enter_context(tc.tile_pool(name="rp", bufs=2))
        opool = ctx.enter_context(tc.tile_pool(name="op", bufs=2))
        rbcpool = ctx.enter_context(tc.tile_pool(name="rbc", bufs=2))
        reppool = ctx.enter_context(tc.tile_pool(name="rep", bufs=2))

        psA = ctx.enter_context(tc.tile_pool(name="psA", bufs=1, space="PSUM"))
        psB = ctx.enter_context(tc.tile_pool(name="psB", bufs=1, space="PSUM"))
        ps_yt = ctx.enter_context(tc.tile_pool(name="psyt", bufs=2, space="PSUM"))
        ps_r = ctx.enter_context(tc.tile_pool(name="psr", bufs=1, space="PSUM"))

        # ---- constants ----
        w1r = w1_d.rearrange("(c p) k -> p c k", p=P)
        w1sb = []
        for c in range(FC):
            t = const.tile([P, HID], bf16, name=f"w1c{c}")
            nc.gpsimd.dma_start(t, w1r[:, c, :])
            w1sb.append(t)
        b1sb = const.tile([P, KC], f32)
        nc.gpsimd.dma_start(b1sb, b1_d[:, :])
        w2sb = const.tile([P, KC, F], bf16)
        wvsb = const.tile([P, FC, F], bf16)
        wosb = const.tile([P, FC, F], bf16)
        b2sb = const.tile([P, FC], f32)
        bosb = const.tile([P, F], f32)
        identsb = const.tile([P, P], f32)

        def mid_consts():
            nc.gpsimd.dma_start(w2sb, w2_d.rearrange("(c p) f -> p c f", p=P))
            nc.gpsimd.dma_start(b2sb, b2_d[:, :])

        def late_consts():
            nc.gpsimd.dma_start(wvsb, wv_d.rearrange("(c p) f -> p c f", p=P))
            nc.gpsimd.dma_start(identsb, id_d[:, :])

        def last_consts():
            nc.gpsimd.dma_start(wosb, wo_d.rearrange("(c p) f -> p c f", p=P))
            nc.gpsimd.dma_start(bosb, bo_d[:, :])

        # ---- inputs ----
        qTsb = big.tile([P, FC, S], bf16, tag="qx")
        qr = q_d.rearrange("(c p) s -> p c s", p=P)
        nc.sync.dma_start(qTsb[:, :, :512], qr[:, :, :512])
        nc.sync.dma_start(qTsb[:, :, 512:], qr[:, :, 512:])
        valTsb = big.tile([P, FC, S], bf16, tag="vT")
        nc.sync.dma_start(valTsb, v_d.rearrange("(c p) s -> p c s", p=P))
        mid_consts()

        hTsb = big.tile([P, KC, S], bf16, tag="hT")
        awTsb = big.tile([P, FC, S], bf16, tag="awT")
        vsb = big.tile([P, SC, F], bf16, tag="v")
        yTsb = big.tile([P, FC, S], bf16, tag="qx")  # reuses qT slot

        e_tiles = {}
        rsums = [None] * NH
        rinvs = [None] * NH
        rbcs = [None] * NH

        # ---- mlp1: hT = relu(w1^T @ qT + b1), with mlp2(fc0)'s
        #      contraction chunks pipelined in as hT rows complete ----
        m2ps = {}

        def mlp2_fill_mms(c):
            if c == 0:
                m2ps[0] = ps_r.tile([P, S], f32, tag="psr", name="m2f")
            for n in range(NS):
                nc.tensor.matmul(
                    m2ps[0][:, n * 512:(n + 1) * 512],
                    w2sb[:, c, 0:P],
                    hTsb[:, c, n * 512:(n + 1) * 512],
                    start=(c == 0),
                    stop=(c == KC - 1),
                )

        for k in range(KC):
            ps = (psA if k % 2 == 0 else psB).tile([P, S], f32, tag="ps")
            for n in range(NS):
                for c in range(FC):
                    nc.tensor.matmul(
                        ps[:, n * 512:(n + 1) * 512],
                        w1sb[c][:, k * P:(k + 1) * P],
                        qTsb[:, c, n * 512:(n + 1) * 512],
                        start=(c == 0),
                        stop=(c == FC - 1),
                    )
            nc.vector.tensor_scalar(
                hTsb[:, k, :], ps, b1sb[:, k:k + 1], 0.0, ALU.add, ALU.max,
            )
            if k >= 1:
                mlp2_fill_mms(k - 1)
            if k == 1:
                late_consts()
            if k == 4:
                last_consts()
        mlp2_fill_mms(KC - 1)
        nc.vector.tensor_scalar_add(awTsb[:, 0, :], m2ps[0], b2sb[:, 0:1])

        # ---- mlp2 chunk (one head pair's features) as filler items ----
        def mlp2_items(fc):
            items = []
            state = {}

            def mk(n, c):
                def go():
                    if n == 0 and c == 0:
                        state[0] = ps_r.tile([P, S], f32, tag="psr", name="m2p")
                    nc.tensor.matmul(
                        state[0][:, n * 512:(n + 1) * 512],
                        w2sb[:, c, fc * P:(fc + 1) * P],
                        hTsb[:, c, n * 512:(n + 1) * 512],
                        start=(c == 0),
                        stop=(c == KC - 1),
                    )
                return go

            for n in range(NS):
                for c in range(KC):
                    items.append(mk(n, c))

            def fin():
                nc.vector.tensor_scalar_add(
                    awTsb[:, fc, :], state[0], b2sb[:, fc:fc + 1],
                )

            items.append(fin)
            return items

        # ---- v projection chunk ----
        def vproj_items(m):
            state = {}

            def mk(c):
                def go():
                    if c == 0:
                        state[0] = ps_yt.tile([P, F], f32, tag="yt", name="vpp")
                    nc.tensor.matmul(
                        state[0],
                        valTsb[:, c, m * P:(m + 1) * P],
                        wvsb[:, c, :],
                        start=(c == 0),
                        stop=(c == FC - 1),
                    )
                    if c == FC - 1:
                        nc.vector.tensor_copy(vsb[:, m, :], state[0])
                return go

            return [mk(c) for c in range(FC)]

        # ---- rbc: per-column 1/rowsum blocks via PE transpose; bounced
        #      to SBUF because DVE can't read two PSUM operands ----
        rbc_ps = [None] * NH

        def rbc_items(q):
            items = []

            reps = {}

            def recip():
                nc.vector.reciprocal(rinvs[q], rsums[q])
                # replicate to [q', m, hb, dk] so each transpose lhsT is a
                # contiguous single-free-dim [128, 128] block
                reps[0] = reppool.tile([P, SC, 2, DK], f32, name="rep")
                nc.vector.tensor_copy(
                    reps[0],
                    rinvs[q][:, :, :].rearrange("p h m -> p m h")
                    .broadcast_to((P, SC, 2, DK)),
                )
                rbc_ps[q] = ps_r.tile([P, S], f32, tag="psr", name="rbcp")

            items.append(recip)

            def mk(m):
                def go():
                    nc.tensor.transpose(
                        rbc_ps[q][:, m * P:(m + 1) * P],
                        reps[0][:, m, :, :],
                        identsb[:, :],
                    )
                return go

            items.extend(mk(m) for m in range(SC))

            def bounce():
                rbcs[q] = rbcpool.tile([P, S], bf16, tag="rbc", name="rbcs")
                nc.vector.tensor_copy(rbcs[q], rbc_ps[q])

            items.append(bounce)
            return items

        # ---- attn_v for pair q, one n-chunk: col-tiled A|B ----
        def attn_v_items(q, n):
            state = {}
            items = []

            def mk(c):
                def go():
                    eA, eB = e_tiles[(q, 0)], e_tiles[(q, 1)]
                    if c == 0:
                        state[0] = ps_yt.tile([P, F], f32, tag="yt", name="avp")
                    nc.tensor.matmul(
                        state[0][0:DK, :],
                        vsb[:, c, (2 * q) * DK:(2 * q + 1) * DK],
                        eA[:, c, n * 512:(n + 1) * 512],
                        start=(c == 0),
                        stop=(c == SC - 1),
                    )
                    nc.tensor.matmul(
                        state[0][DK:P, :],
                        vsb[:, c, (2 * q + 1) * DK:(2 * q + 2) * DK],
                        eB[:, c, n * 512:(n + 1) * 512],
                        start=(c == 0),
                        stop=(c == SC - 1),
                    )
                return go

            items.extend(mk(c) for c in range(SC))

            def fin():
                nc.vector.tensor_mul(
                    yTsb[:, q, n * 512:(n + 1) * 512],
                    state[0],
                    rbcs[q][:, n * 512:(n + 1) * 512],
                )

            items.append(fin)
            return items

        # ---- scores + exp slot for pair q, token tile m ----
        def scores_slot(q, m, filler, per_slot):
            eA, eB = e_tiles[(q, 0)], e_tiles[(q, 1)]
            a_l = awTsb[0:DK, q, m * P:(m + 1) * P]
            b_l = awTsb[DK:P, q, m * P:(m + 1) * P]
            psa = psA.tile([P, S], f32, tag="ps")
            psb = psB.tile([P, S], f32, tag="ps")
            for n in range(NS):
                nc.tensor.matmul(
                    psa[:, n * 512:(n + 1) * 512],
                    a_l,
                    awTsb[0:DK, q, n * 512:(n + 1) * 512],
                    start=True,
                    stop=True,
                )
                nc.tensor.matmul(
                    psb[:, n * 512:(n + 1) * 512],
                    b_l,
                    awTsb[DK:P, q, n * 512:(n + 1) * 512],
                    start=True,
                    stop=True,
                )
            nc.scalar.activation(eA[:, m, :], psa, AF.Exp, scale=scale,
                                 accum_out=rsums[q][:, 0, m:m + 1])
            nc.scalar.activation(eB[:, m, :], psb, AF.Exp, scale=scale,
                                 accum_out=rsums[q][:, 1, m:m + 1])
            for _ in range(per_slot):
                if filler:
                    filler.pop(0)()

        def run_pair(q, filler, pinned=None):
            e_tiles[(q, 0)] = epool.tile([P, SC, S], bf16, tag="e", name="eA")
            e_tiles[(q, 1)] = epool.tile([P, SC, S], bf16, tag="e", name="eB")
            rsums[q] = rpool.tile([P, 2, SC], f32, tag="rs", name="rs")
            rinvs[q] = rpool.tile([P, 2, SC], f32, tag="ri", name="ri")
            per_slot = (len(filler) + SC - 1) // SC
            for m in range(SC):
                scores_slot(q, m, filler, per_slot)
                for go in (pinned or {}).get(m, []):
                    go()
            while filler:
                filler.pop(0)()

        # ---- attn_v block for pair q: n0 MMs | filler | rbc | scaled
        #      copies.  `mid` runs between the MM chains so the rbc recip
        #      (gated on the rowsum reduces) lands late enough not to
        #      stall the PE queue.
        def av_block(q, mid):
            a0 = attn_v_items(q, 0)
            a1 = attn_v_items(q, 1)
            return a0[:-1] + mid + rbc_items(q) + [a0[-1]] + a1

        # ---- pipeline: pair q scores/exp | attn_v(q-1) + upcoming
        #      mlp2/vproj as PE filler ----
        run_pair(0, mlp2_items(1) + vproj_items(0) + vproj_items(1)
                 + vproj_items(2) + vproj_items(3) + vproj_items(4)
                 + vproj_items(5) + vproj_items(6) + vproj_items(7))
        run_pair(1, av_block(0, mlp2_items(2)))
        # half out-projection: contract feature pairs [c0, c0+1] for token
        # tile m.  First half adds the bias into o1; second half adds o1
        # and streams the result out.
        o1_sb = big.tile([P, SC, F], f32, tag="o1")

        def final_items(c0, m):
            def go():
                o_p = ps_yt.tile([P, F], f32, tag="yt", name="op")
                for c in (c0, c0 + 1):
                    nc.tensor.matmul(
                        o_p,
                        yTsb[:, c, m * P:(m + 1) * P],
                        wosb[:, c, :],
                        start=(c == c0),
                        stop=(c == c0 + 1),
                    )
                if c0 == 0:
                    nc.vector.tensor_add(o1_sb[:, m, :], o_p, bosb)
                else:
                    o_sb = opool.tile([P, F], f32, tag="o", name="o2")
                    nc.vector.tensor_add(o_sb, o_p, o1_sb[:, m, :])
                    nc.sync.dma_start(out_d[m * P:(m + 1) * P, :], o_sb)

            return [go]

        run_pair(2, av_block(1, mlp2_items(3)))
        # pair 3: its own attn_v chains ride the score slots, lagging the
        # exp stream by one token tile, so they finish with it
        a30 = attn_v_items(3, 0)
        a31 = attn_v_items(3, 1)
        pinned = {}
        for c in range(SC):
            pinned.setdefault(max(min(c + 1, 7), 3), []).append(a30[c])
            pinned.setdefault(max(min(c + 1, 7), 6), []).append(a31[c])
        run_pair(3, av_block(2, []), pinned)

        # ---- tail: pair-3 rbc + scaled copies, then the out-projection --
        for go in rbc_items(3) + [a30[-1], a31[-1]]:
            go()
        for m in range(SC):
            final_items(0, m)[0]()
            final_items(2, m)[0]()

    nc.compile()
    return nc


def _get_nc():
    global _CACHED_NC
    if _CACHED_NC is None:
        _CACHED_NC = _build_nc()
    return _CACHED_NC


def _make_in_maps(inputs):
    query = np.asarray(inputs["query"], np.float32)
    value = np.asarray(inputs["value"], np.float32)
    import ml_dtypes
    bf = ml_dtypes.bfloat16
    w1 = np.asarray(inputs["w1"], np.float32)
    b1 = np.asarray(inputs["b1"], np.float32)
    w2 = np.asarray(inputs["w2"], np.float32)
    b2 = np.asarray(inputs["b2"], np.float32)
    wv = np.asarray(inputs["wv"], np.float32)
    bv = np.asarray(inputs["bv"], np.float32)
    wo = np.asarray(inputs["wo"], np.float32)
    bo = np.asarray(inputs["bo"], np.float32)

    b1r = np.ascontiguousarray(b1.reshape(HID // P, P).T)
    b2r = np.ascontiguousarray(b2.reshape(F // P, P).T)
    # softmax rows sum to 1, so the value bias commutes past attention:
    # out = attn(v @ wv) @ wo + (bv @ wo + bo)
    bo2 = bo + bv @ wo
    bo2b = np.ascontiguousarray(np.broadcast_to(bo2, (P, F)).astype(np.float32))

    shared = dict(w1=w1.astype(bf), w2=w2.astype(bf), wv=wv.astype(bf),
                  wo=wo.astype(bf), b1r=b1r, b2r=b2r, bo2b=bo2b,
                  ident=np.eye(P, dtype=np.float32))
    return [dict(qT=np.ascontiguousarray(query[i].T).astype(bf),
                 vT=np.ascontiguousarray(value[i].T).astype(bf), **shared)
            for i in range(N_CORES)]


def kernel(**inputs):
    in_maps = _make_in_maps(inputs)

    from concourse.bass_utils import run_bass_kernel_spmd

    nc = _get_nc()
    res = run_bass_kernel_spmd(nc, in_maps, core_ids=list(range(N_CORES)))
    out = np.stack([res.results[i]["out"] for i in range(N_CORES)], axis=0)
    return out.astype(np.float32)


if __name__ == "__main__":
    nc = _get_nc()
    print("built ok")

